# revision 7
# baseline (speedup 1.0000x reference)
"""Trainium2 Bass kernel for CapsuleLayer dynamic routing (nn_CapsuleLayer_69002944578111).

Full-input contract: kernel(x, W) takes the full arrays
  x: [64, 2048, 8] f32, W: [1, 2048, 32, 16, 8] f32
and returns squash(s)[64, 32, 16] f32 matching reference().

Sharding: input-capsule axis I=2048 split across 8 cores (256 each).
All routing math per (b, i) is local; only the final sum over i crosses
shards, done host-side on the tiny [64, 32, 16] partials.

Key algebra (ROUTINGS=3, b0=0):
  us[b,i,j]   = sum_d u_hat = sum_p x[b,i,p] * Wsum[i,j,p]   (Wsum = sum_d W)
  c1 = softmax_j(us/J);  b2 = us * (1/J + c1);  c2 = softmax_j(b2)
  s[b,j,d]    = sum_{i,p} (c2[b,i,j] * x[b,i,p]) * W[i,j,d,p]
  out = squash(s)
Only the last iteration's output survives in the reference loop, so the
full u_hat tensor is never materialized.
"""

import os
from contextlib import ExitStack

import numpy as np

B, I, P = 64, 2048, 8
J, D = 32, 16
NCORES = 8
IC = I // NCORES  # 256 input capsules per core
EPS = 1e-7

_MODULE_CACHE = {}


def ts(i, size):
    return slice(i * size, (i + 1) * size)


def _bcast_mid(ap2d, count):
    """[P, N] AP -> [P, count, N] AP with a stride-0 middle dim."""
    import concourse.bass as bass

    return bass.AP(
        tensor=ap2d.tensor,
        offset=ap2d.offset,
        ap=[ap2d.ap[0], [0, count], ap2d.ap[-1]],
    )


def build_module():
    """Build the (core-agnostic) Bass/Tile module. Same NEFF runs on all 8
    cores; per-core data differences come entirely from the host-sliced
    inputs."""
    import concourse.bass as bass
    import concourse.bacc as bacc
    import concourse.tile as tile
    from concourse import mybir

    f32 = mybir.dt.float32
    nc = bacc.Bacc("TRN2", target_bir_lowering=False)

    # DRAM I/O (per-core shard layouts, host-prepared):
    #   xp  [8,  IC*B ]  xp[p, i*64+b]          = x[b, i, p]
    #   ws  [8,  IC*J ]  ws[p, i*32+j]          = sum_d W[i, j, d, p]
    #   xt  [128, 2*8*B] xt[q, ((h*8)+p)*64+b]  = x[b, 128h+q, p]
    #   wt  [128, 2*J*8*D] wt[q, h, j, p, d]    = W[128h+q, j, d, p]
    xp_d = nc.dram_tensor("xp", [P, IC * B], f32, kind="ExternalInput")
    ws_d = nc.dram_tensor("ws", [P, IC * J], f32, kind="ExternalInput")
    xt_d = nc.dram_tensor("xt", [128, 2 * P * B], f32, kind="ExternalInput")
    wt_d = nc.dram_tensor("wt", [128, 2 * J * P * D], f32, kind="ExternalInput")
    id_d = nc.dram_tensor("ident", [B, B], f32, kind="ExternalInput")
    out_d = nc.dram_tensor("out", [B, J * D], f32, kind="ExternalOutput")

    with ExitStack() as ctx:
        tc = ctx.enter_context(tile.TileContext(nc))
        sing = ctx.enter_context(tc.tile_pool(name="sing", bufs=1))
        xpool = ctx.enter_context(tc.tile_pool(name="xpool", bufs=3))
        chain = ctx.enter_context(tc.tile_pool(name="chain", bufs=2))
        dens = ctx.enter_context(tc.tile_pool(name="dens", bufs=4))
        ypool = ctx.enter_context(tc.tile_pool(name="ypool", bufs=2))
        ps_us = ctx.enter_context(tc.tile_pool(name="ps_us", bufs=3, space="PSUM"))
        ps_t = ctx.enter_context(tc.tile_pool(name="ps_t", bufs=2, space="PSUM"))
        ps_s = ctx.enter_context(tc.tile_pool(name="ps_s", bufs=1, space="PSUM"))

        # ---- constants / full-lifetime tiles ----
        id_sb = sing.tile([B, B], f32)
        nc.sync.dma_start(out=id_sb[:], in_=id_d[:])
        xt_sb = sing.tile([128, 2, P, B], f32)
        nc.sync.dma_start(out=xt_sb[:], in_=xt_d[:].rearrange("q (h p b) -> q h p b", h=2, p=P))
        wt_sb = sing.tile([128, 2, J, P, D], f32)
        nc.sync.dma_start(
            out=wt_sb[:], in_=wt_d[:].rearrange("q (h j p d) -> q h j p d", h=2, j=J, p=P)
        )

        us_sb = sing.tile([B, J, IC], f32)  # us^T-source: [b, j, i]
        usT_sb = sing.tile([128, 2, J, B], f32)  # [q, h, j, b]
        e1_sb = sing.tile([128, 2, J, B], f32)
        c2_sb = sing.tile([128, 2, J, B], f32)

        # ---- phase A: us[b,i,j] via 256 matmuls (K=8, M=64, N=32) ----
        NI_CHUNK = 16  # i per DMA chunk = one PSUM bank group
        for icnk in range(IC // NI_CHUNK):
            xp_c = xpool.tile([P, NI_CHUNK * B], f32, tag="xp")
            nc.sync.dma_start(out=xp_c[:], in_=xp_d[:, ts(icnk, NI_CHUNK * B)])
            ws_c = xpool.tile([P, NI_CHUNK * J], f32, tag="ws")
            nc.sync.dma_start(out=ws_c[:], in_=ws_d[:, ts(icnk, NI_CHUNK * J)])
            us_ps = ps_us.tile([B, 16, J], f32, tag="usps")
            for ii in range(16):
                nc.tensor.matmul(
                    us_ps[:, ii, :],
                    lhsT=xp_c[:, ts(ii, B)],
                    rhs=ws_c[:, ts(ii, J)],
                    start=True,
                    stop=True,
                )
            i0 = icnk * NI_CHUNK
            # scatter bank -> us_sb[b, j, i0:i0+16]
            nc.scalar.copy(
                out=us_sb[:, :, i0 : i0 + 16],
                in_=us_ps[:].rearrange("b i j -> b j i"),
            )

        # ---- phase B: PE-transpose us -> usT [i(q), h, j, b]; e1 = exp(us/J) ----
        for h in range(2):
            for jg in range(J // 8):
                tps = ps_t.tile([128, 8, B], f32, tag="tps")
                for jj in range(8):
                    j = jg * 8 + jj
                    nc.tensor.transpose(
                        tps[:, jj, :], us_sb[:, j, ts(h, 128)], id_sb[:]
                    )
                nc.scalar.copy(out=usT_sb[:, h, ts(jg, 8), :], in_=tps[:])
                nc.scalar.activation(
                    out=e1_sb[:, h, ts(jg, 8), :],
                    in_=tps[:],
                    func=mybir.ActivationFunctionType.Exp,
                    scale=1.0 / J,
                )

        # ---- phase C: softmax chain -> c2 ----
        for h in range(2):
            den1 = dens.tile([128, B], f32, tag="den")
            nc.vector.reduce_sum(
                den1[:],
                e1_sb[:, h].rearrange("q j b -> q b j"),
                axis=mybir.AxisListType.X,
            )
            r1 = dens.tile([128, B], f32, tag="rec")
            nc.vector.reciprocal(r1[:], den1[:])
            c1 = chain.tile([128, J, B], f32, tag="c1")
            nc.vector.tensor_mul(c1[:], e1_sb[:, h], _bcast_mid(r1[:], J))
            b2 = chain.tile([128, J, B], f32, tag="b2")
            nc.vector.scalar_tensor_tensor(
                out=b2[:],
                in0=c1[:],
                scalar=1.0 / J,
                in1=usT_sb[:, h],
                op0=mybir.AluOpType.add,
                op1=mybir.AluOpType.mult,
            )
            e2 = chain.tile([128, J, B], f32, tag="c1")
            nc.scalar.activation(
                out=e2[:], in_=b2[:], func=mybir.ActivationFunctionType.Exp
            )
            den2 = dens.tile([128, B], f32, tag="den")
            nc.vector.reduce_sum(
                den2[:], e2[:].rearrange("q j b -> q b j"), axis=mybir.AxisListType.X
            )
            r2 = dens.tile([128, B], f32, tag="rec")
            nc.vector.reciprocal(r2[:], den2[:])
            nc.vector.tensor_mul(c2_sb[:, h], e2[:], _bcast_mid(r2[:], J))

        # ---- phase D: Y = c2 * x ; s += Y^T @ W  (accumulate over p, h) ----
        # One PSUM bank holds all 32 j-regions. start=True lazily zeroes the
        # whole 2KB zero-region, so only the very first matmul starts the
        # group and only the very last stops it; every region's first touch
        # materializes the zero (overwrite), later touches accumulate.
        ps_out = ps_s.tile([B, J, D], f32)
        n_mm = P * 2 * J
        mm = 0
        for p in range(P):
            for h in range(2):
                yp = ypool.tile([128, J, B], f32, tag="yp")
                nc.vector.tensor_mul(
                    yp[:], c2_sb[:, h], _bcast_mid(xt_sb[:, h, p, :], J)
                )
                for j in range(J):
                    nc.tensor.matmul(
                        ps_out[:, j, :],
                        lhsT=yp[:, j, :],
                        rhs=wt_sb[:, h, j, p, :],
                        start=(mm == 0),
                        stop=(mm == n_mm - 1),
                    )
                    mm += 1

        # ---- phase E: write out ----
        out_sb = sing.tile([B, J * D], f32)
        nc.vector.tensor_copy(out_sb[:], ps_out[:].rearrange("b j d -> b (j d)"))
        nc.sync.dma_start(out=out_d[:], in_=out_sb[:])

    nc.compile()
    return nc


def make_in_maps(x, W):
    """Host-side shard + relayout. Returns list of 8 per-core input dicts."""
    x = np.ascontiguousarray(np.asarray(x, dtype=np.float32))
    W = np.ascontiguousarray(np.asarray(W, dtype=np.float32))
    W0 = W[0]  # [I, J, D, P]
    Wsum = W0.sum(axis=2)  # [I, J, P]
    ident = np.eye(B, dtype=np.float32)
    in_maps = []
    for c in range(NCORES):
        sl = slice(c * IC, (c + 1) * IC)
        xc = x[:, sl, :]  # [B, IC, P]
        Wc = W0[sl]  # [IC, J, D, P]
        wsc = Wsum[sl]  # [IC, J, P]
        xp = np.ascontiguousarray(xc.transpose(2, 1, 0)).reshape(P, IC * B)
        ws = np.ascontiguousarray(wsc.transpose(2, 0, 1)).reshape(P, IC * J)
        xt = np.ascontiguousarray(
            xc.transpose(1, 2, 0).reshape(2, 128, P, B).transpose(1, 0, 2, 3)
        ).reshape(128, 2 * P * B)
        wt = np.ascontiguousarray(
            Wc.transpose(0, 1, 3, 2).reshape(2, 128, J, P, D).transpose(1, 0, 2, 3, 4)
        ).reshape(128, 2 * J * P * D)
        in_maps.append({"xp": xp, "ws": ws, "xt": xt, "wt": wt, "ident": ident})
    return in_maps


def finalize(partials):
    """Sum per-core partial s and apply squash."""
    s = np.zeros((B, J * D), dtype=np.float32)
    for pt in partials:
        s = s + pt.astype(np.float32)
    s = s.reshape(B, J, D)
    s2 = np.sum(s * s, axis=-1, keepdims=True, dtype=np.float32)
    scale = s2 / (1.0 + s2) / np.sqrt(s2 + EPS)
    return (scale * s).astype(np.float32)


def _get_module():
    if "nc" not in _MODULE_CACHE:
        _MODULE_CACHE["nc"] = build_module()
    return _MODULE_CACHE["nc"]


def run_on_hw(x, W, trace=False):
    """Run on the 8 NeuronCores; returns (output, BassKernelResults)."""
    from concourse.bass_utils import run_bass_kernel_spmd

    nc = _get_module()
    in_maps = make_in_maps(x, W)
    res = run_bass_kernel_spmd(
        nc, in_maps, core_ids=list(range(NCORES)), trace=trace
    )
    out = finalize([r["out"] for r in res.results])
    return out, res


def kernel(**inputs):
    out, _ = run_on_hw(inputs["x"], inputs["W"])
    return out
